# revision 1
# baseline (speedup 1.0000x reference)
"""Trainium2 Bass kernel for an AttentionBlock with a single KV token.

Math: with kv_len == 1 the softmax over the key axis is identically 1.0,
so the attention output for every query position equals v, and the
LayerNorm / q-projection never influence the output:

    kv      = cond_emb @ kv_w.T + kv_b          # (b, 2c)
    v_in    = kv[:, c:]                         # (b, c)
    v_full  = v_in @ wv.T + bv                  # (b, c)   wv = in_proj_w[2c:]
    av      = v_full @ out_w.T + out_b          # (b, c)
    y       = x + av[:, :, None, None]          # (b, c, h, w)

This is a tiny per-batch matmul chain plus one huge memory-bound
broadcast add.  Sharding: data-parallel over batch (8 batches/core),
weights replicated (host pre-transposed into matmul layouts).

Per core: 33.55 MB in + 33.55 MB out + 1.07 MB consts.  The kernel is
pure DMA-roofline: loads stream on the sync HWDGE ring, stores on the
scalar HWDGE ring (sum sustains ~425 GB/s, the SBUF AXI fabric limit),
broadcast-adds run in-place on DVE (2x fp32 tensor_scalar mode, hidden
under DMA).  First/last row-tiles are quartered to speed ramp-up and
shorten the final load->add->store pipeline tail; a few tail stores are
routed onto the sync ring so both rings stay busy to the end.
Measured ~172-174 us/core quiet, ~200 us with both stack-mate cores
fully overlapped (716 GB/s HBM stack shared per core pair) -- both at
the respective memory roofline.
"""

import numpy as np

import concourse.bacc as bacc
import concourse.mybir as mybir
from concourse.bass_utils import run_bass_kernel_spmd
from concourse.tile import TileContext

B, C, H, W = 64, 256, 64, 64
EMB = 512
HWD = H * W               # 4096
NCORES = 8
BS = B // NCORES          # 8 batches per core
ROWS = BS * C             # 2048 rows of length HW per core
NT = ROWS // 128          # 16 tiles of [128, 4096]
F32 = mybir.dt.float32

_CACHE = {}


# Column offsets inside the packed consts tensor [128, CONST_COLS]:
#   cond:  [p, e*8 + b]        = cond_emb[b, 128e + p]           (32 cols)
#   kvw:   [p, e*256 + j]      = kv_w[256 + j, 128e + p]         (1024 cols)
#   wv:    [p, i*256 + j]      = in_proj_w[512 + j, 128i + p]    (512 cols)
#   outw:  [p, j*256 + c]      = out_w[c, 128j + p]              (512 cols)
#   bias:  [p, u*3 + k]; k=0: kv_b[256+u*128+p],
#          k=1: in_proj_b[512+u*128+p], k=2: out_b[u*128+p]      (6 cols)
COND_O = 0
KVW_O = COND_O + 4 * BS
WV_O = KVW_O + 4 * C
OUTW_O = WV_O + 2 * C
BIAS_O = OUTW_O + 2 * C
CONST_COLS = BIAS_O + 6


def _build_nc():
    nc = bacc.Bacc("TRN2", target_bir_lowering=False, debug=False)

    x_d = nc.dram_tensor("x", [ROWS, HWD], F32, kind="ExternalInput").ap()
    consts_d = nc.dram_tensor("consts", [128, CONST_COLS], F32, kind="ExternalInput").ap()
    y_d = nc.dram_tensor("y", [ROWS, HWD], F32, kind="ExternalOutput").ap()

    with TileContext(nc) as tc:
        with (
            tc.tile_pool(name="const", bufs=1) as cpool,
            tc.tile_pool(name="psum", bufs=2, space="PSUM") as ppool,
            tc.tile_pool(name="small", bufs=2) as spool,
            tc.tile_pool(name="xio", bufs=10) as xpool,
            tc.tile_pool(name="xhalf", bufs=4) as hpool,
        ):
            csb = cpool.tile([128, CONST_COLS], F32, tag="consts")
            # Head of the scalar HWDGE ring: stores don't exist for the
            # first ~14us, so this costs nothing and keeps the sync ring
            # free to start streaming x immediately.
            nc.scalar.dma_start(out=csb[:], in_=consts_d[:])
            cond_sb = csb[:, COND_O : COND_O + 4 * BS]
            kvw_sb = csb[:, KVW_O : KVW_O + 4 * C]
            wv_sb = csb[:, WV_O : WV_O + 2 * C]
            outw_sb = csb[:, OUTW_O : OUTW_O + 2 * C]
            bias_sb = csb[:, BIAS_O : BIAS_O + 6]

            # v_inT[u][p, b] = kv[b, 256 + u*128 + p]
            vin_sb = [spool.tile([128, BS], F32, tag=f"vin{u}", name=f"vin{u}") for u in range(2)]
            for u in range(2):
                pv = ppool.tile([128, BS], F32)
                for e in range(4):
                    nc.tensor.matmul(
                        out=pv[:],
                        lhsT=kvw_sb[:, e * C + u * 128 : e * C + u * 128 + 128],
                        rhs=cond_sb[:, e * BS : (e + 1) * BS],
                        start=(e == 0),
                        stop=(e == 3),
                    )
                nc.vector.tensor_scalar_add(
                    out=vin_sb[u][:], in0=pv[:], scalar1=bias_sb[:, 0 + u * 3 : 1 + u * 3]
                )

            # v_fullT[u][p, b] = v_full[b, u*128 + p]
            vf_sb = [spool.tile([128, BS], F32, tag=f"vf{u}", name=f"vf{u}") for u in range(2)]
            for u in range(2):
                pv = ppool.tile([128, BS], F32)
                for i in range(2):
                    nc.tensor.matmul(
                        out=pv[:],
                        lhsT=wv_sb[:, i * C + u * 128 : i * C + u * 128 + 128],
                        rhs=vin_sb[i][:],
                        start=(i == 0),
                        stop=(i == 1),
                    )
                nc.vector.tensor_scalar_add(
                    out=vf_sb[u][:], in0=pv[:], scalar1=bias_sb[:, 1 + u * 3 : 2 + u * 3]
                )

            # avT[u][p, b] = av[b, u*128 + p]
            av_sb = [spool.tile([128, BS], F32, tag=f"av{u}", name=f"av{u}") for u in range(2)]
            for u in range(2):
                pv = ppool.tile([128, BS], F32)
                for j in range(2):
                    nc.tensor.matmul(
                        out=pv[:],
                        lhsT=outw_sb[:, j * C + u * 128 : j * C + u * 128 + 128],
                        rhs=vf_sb[j][:],
                        start=(j == 0),
                        stop=(j == 1),
                    )
                nc.vector.tensor_scalar_add(
                    out=av_sb[u][:], in0=pv[:], scalar1=bias_sb[:, 2 + u * 3 : 3 + u * 3]
                )

            # Stream x: row r = b*256 + c ; tile t covers rows [128t, 128t+128)
            # -> batch b = t//2, channel c = (t%2)*128 + p, scalar = av_sb[t%2][p, t//2]
            def add_store(tile_ap, dram_rows, av_ap, store_eng):
                # Broadcast-add on DVE (2x mode, ~2.8us/full tile) in-place.
                nc.vector.tensor_scalar_add(out=tile_ap, in0=tile_ap, scalar1=av_ap)
                store_eng.dma_start(out=dram_rows, in_=tile_ap)

            # Stores default to the scalar HWDGE ring; the tail stores
            # alternate onto the sync ring (empty once loads finish) so the
            # stores-only end phase runs dual-row at full DMA rate.
            HH = HWD // 2
            tail_stores = []
            for t in range(NT):
                u, b = t % 2, t // 2
                av_ap = av_sb[u][:, b : b + 1]
                rows = slice(t * 128, (t + 1) * 128)
                if t in (0, NT - 1):
                    # Quarter the first tile (small first DMAs ramp the SDMA
                    # engines faster, stores start sooner) and the last tile
                    # (short load->add->store pipeline tail after the final
                    # load, final stores split across both rings).
                    QQ = HWD // 4
                    for h in range(4):
                        quar = hpool.tile([128, QQ], F32, tag="xq", name=f"xq{t}_{h}")
                        cols = slice(h * QQ, (h + 1) * QQ)
                        nc.sync.dma_start(out=quar[:], in_=x_d[rows, cols])
                        if t == NT - 1 and h == 2:
                            nc.vector.tensor_scalar_add(
                                out=quar[:], in0=quar[:], scalar1=av_ap
                            )
                            tail_stores.append((y_d[rows, cols], quar[:]))
                        else:
                            add_store(quar[:], y_d[rows, cols], av_ap, nc.scalar)
                elif t in (12, 14):
                    # Split this store across the rings: first half to the
                    # scalar ring now, second half to the sync-ring tail.
                    tile = xpool.tile([128, HWD], F32, tag="xt")
                    nc.sync.dma_start(out=tile[:], in_=x_d[rows, :])
                    nc.vector.tensor_scalar_add(out=tile[:], in0=tile[:], scalar1=av_ap)
                    nc.scalar.dma_start(out=y_d[rows, 0:HH], in_=tile[:, 0:HH])
                    tail_stores.append((y_d[rows, HH:], tile[:, HH:]))
                else:
                    tile = xpool.tile([128, HWD], F32, tag="xt")
                    nc.sync.dma_start(out=tile[:], in_=x_d[rows, :])
                    add_store(tile[:], y_d[rows, :], av_ap, nc.scalar)
            # Issued after every load in program order -> they sit at the end
            # of the sync ring FIFO and never block a load.
            for dst, src in tail_stores:
                nc.sync.dma_start(out=dst, in_=src)

    nc.compile()
    return nc


def _prep_consts(in_proj_w, in_proj_b, out_w, out_b, kv_w, kv_b):
    c = C
    base = np.empty((128, CONST_COLS), np.float32)
    base[:, KVW_O : KVW_O + 4 * c] = (
        kv_w[c : 2 * c, :].T.reshape(4, 128, c).transpose(1, 0, 2).reshape(128, 4 * c)
    )
    base[:, WV_O : WV_O + 2 * c] = (
        in_proj_w[2 * c :, :].T.reshape(2, 128, c).transpose(1, 0, 2).reshape(128, 2 * c)
    )
    base[:, OUTW_O : OUTW_O + 2 * c] = (
        out_w.T.reshape(2, 128, c).transpose(1, 0, 2).reshape(128, 2 * c)
    )
    for u in range(2):
        base[:, BIAS_O + u * 3 + 0] = kv_b[c + u * 128 : c + (u + 1) * 128]
        base[:, BIAS_O + u * 3 + 1] = in_proj_b[2 * c + u * 128 : 2 * c + (u + 1) * 128]
        base[:, BIAS_O + u * 3 + 2] = out_b[u * 128 : (u + 1) * 128]
    return base


def make_in_maps(x, cond_emb, in_proj_w, in_proj_b, out_w, out_b, kv_w, kv_b):
    base = _prep_consts(in_proj_w, in_proj_b, out_w, out_b, kv_w, kv_b)
    in_maps = []
    for r in range(NCORES):
        xs = np.ascontiguousarray(
            x[r * BS : (r + 1) * BS].reshape(ROWS, HWD), dtype=np.float32
        )
        consts = base.copy()
        consts[:, COND_O : COND_O + 4 * BS] = (
            cond_emb[r * BS : (r + 1) * BS]
            .T.reshape(4, 128, BS)
            .transpose(1, 0, 2)
            .reshape(128, 4 * BS)
        )
        in_maps.append({"x": xs, "consts": consts})
    return in_maps


def get_nc():
    if "nc" not in _CACHE:
        _CACHE["nc"] = _build_nc()
    return _CACHE["nc"]


def kernel(x, cond_emb, ln_gamma, ln_beta, in_proj_w, in_proj_b, out_w, out_b, kv_w, kv_b):
    x = np.asarray(x, dtype=np.float32)
    nc = get_nc()
    in_maps = make_in_maps(
        x,
        np.asarray(cond_emb, np.float32),
        np.asarray(in_proj_w, np.float32),
        np.asarray(in_proj_b, np.float32),
        np.asarray(out_w, np.float32),
        np.asarray(out_b, np.float32),
        np.asarray(kv_w, np.float32),
        np.asarray(kv_b, np.float32),
    )
    res = run_bass_kernel_spmd(nc, in_maps, core_ids=list(range(NCORES)))
    y = np.empty((B, C, H, W), np.float32)
    for r in range(NCORES):
        y[r * BS : (r + 1) * BS] = res.results[r]["y"].reshape(BS, C, H, W)
    return y



# revision 2
# speedup vs baseline: 1.1463x; 1.1463x over previous
"""Trainium2 Bass kernel for an AttentionBlock with a single KV token.

Math: with kv_len == 1 the softmax over the key axis is identically 1.0,
so the attention output for every query position equals v, and the
LayerNorm / q-projection never influence the output.  The whole weight
chain folds on the host (pure weight preprocessing):

    M      = out_w @ wv @ kv_w[c:2c]          # (c, emb), wv = in_proj_w[2c:]
    bconst = out_w @ (wv @ kv_b[c:] + bv) + out_b
    av     = cond_emb @ M.T + bconst          # (b, c)  one on-device matmul
    y      = x + av[:, :, None, None]         # (b, c, h, w)

This is a tiny per-batch matmul plus one huge memory-bound broadcast
add.  Sharding: data-parallel over batch (8 batches/core), weights
replicated.

Per core the DMA traffic is 33.55 MB in + 33.55 MB out + 0.27 MB consts.
A TRN2 core sustains a flat ~400-425 GB/s of total DMA traffic in any
direction/mix (16 shared DMA engines; x8 cores = device HBM bandwidth),
so the kernel is pinned at bytes/throat + ~7.3 us runtime preamble +
~2 us teardown ~= 171.5 us.  Every scheduling alternative (bulk-phased,
3-queue, casting DMAs, giant descriptors) measures identically; the only
lever that moved the needle was shrinking the consts DMA: M and cond
ride in bf16 (av |err| ~1e-4 vs the 2e-2 tolerance; the x path stays
exact fp32), cutting consts from 1.07 MB to 0.27 MB (-1.4 us measured
in an interleaved A/B: quiet-run floor 171.4 us vs 172.7 us).

Streaming schedule: loads on the sync HWDGE ring, stores on the scalar
HWDGE ring, broadcast-adds in-place on DVE (hidden under DMA).
First/last row-tiles are quartered to speed ramp-up and shorten the
final load->add->store tail; a few tail stores are routed onto the sync
ring so both rings stay busy to the end.
"""

import numpy as np

import concourse.bacc as bacc
import concourse.mybir as mybir
from concourse.bass_utils import run_bass_kernel_spmd
from concourse.tile import TileContext

B, C, H, W = 64, 256, 64, 64
EMB = 512
HWD = H * W               # 4096
NCORES = 8
BS = B // NCORES          # 8 batches per core
ROWS = BS * C             # 2048 rows of length HW per core
NT = ROWS // 128          # 16 tiles of [128, 4096]
F32 = mybir.dt.float32

_CACHE = {}


BF16 = mybir.dt.bfloat16

# The whole weight chain folds on the host (pure weight preprocessing):
#   M      = out_w @ wv @ kv_w[c:2c]     (c, emb)
#   bconst = out_w @ (wv @ kv_b[c:] + bv) + out_b
#   av     = cond_emb @ M.T + bconst     -> one on-device matmul stage.
# M and cond ride in bf16 (av |err| ~1e-4 abs vs tolerance 2e-2), so the
# packed consts shrink from 1.07 MB to ~0.27 MB of DMA.
# consts16 [128, 1056] bf16:  cond: [p, e*8+b] = cond_emb[b, 128e+p] (32)
#                             m:    [p, 32 + e*256+j] = M[j, 128e+p] (1024)
# consts32 [128, 2]   fp32:   bconst: [p, u] = bconst[u*128+p]
COND_O = 0
M_O = COND_O + 4 * BS
C16_COLS = M_O + 4 * C


def _build_nc():
    nc = bacc.Bacc("TRN2", target_bir_lowering=False, debug=False)

    x_d = nc.dram_tensor("x", [ROWS, HWD], F32, kind="ExternalInput").ap()
    c16_d = nc.dram_tensor("consts16", [128, C16_COLS], BF16, kind="ExternalInput").ap()
    c32_d = nc.dram_tensor("consts32", [128, 2], F32, kind="ExternalInput").ap()
    y_d = nc.dram_tensor("y", [ROWS, HWD], F32, kind="ExternalOutput").ap()

    with TileContext(nc) as tc:
        with (
            tc.tile_pool(name="const", bufs=1) as cpool,
            tc.tile_pool(name="psum", bufs=2, space="PSUM") as ppool,
            tc.tile_pool(name="small", bufs=2) as spool,
            tc.tile_pool(name="xio", bufs=10) as xpool,
            tc.tile_pool(name="xhalf", bufs=4) as hpool,
        ):
            csb = cpool.tile([128, C16_COLS], BF16, tag="consts")
            bcsb = cpool.tile([128, 2], F32, tag="consts32", name="bcsb")
            # Head of the scalar HWDGE ring: stores don't exist for the
            # first ~14us, so this costs nothing and keeps the sync ring
            # free to start streaming x immediately.
            nc.scalar.dma_start(out=csb[:], in_=c16_d[:])
            nc.scalar.dma_start(out=bcsb[:], in_=c32_d[:])
            cond_sb = csb[:, COND_O : COND_O + 4 * BS]
            m_sb = csb[:, M_O : M_O + 4 * C]

            # avT[u][p, b] = av[b, u*128 + p] ; av = cond @ M.T + bconst
            av_sb = [spool.tile([128, BS], F32, tag=f"av{u}", name=f"av{u}") for u in range(2)]
            for u in range(2):
                pv = ppool.tile([128, BS], F32)
                for e in range(4):
                    nc.tensor.matmul(
                        out=pv[:],
                        lhsT=m_sb[:, e * C + u * 128 : e * C + u * 128 + 128],
                        rhs=cond_sb[:, e * BS : (e + 1) * BS],
                        start=(e == 0),
                        stop=(e == 3),
                    )
                nc.vector.tensor_scalar_add(
                    out=av_sb[u][:], in0=pv[:], scalar1=bcsb[:, u : u + 1]
                )

            # Stream x: row r = b*256 + c ; tile t covers rows [128t, 128t+128)
            # -> batch b = t//2, channel c = (t%2)*128 + p, scalar = av_sb[t%2][p, t//2]
            def add_store(tile_ap, dram_rows, av_ap, store_eng):
                # Broadcast-add on DVE (2x mode, ~2.8us/full tile) in-place.
                nc.vector.tensor_scalar_add(out=tile_ap, in0=tile_ap, scalar1=av_ap)
                store_eng.dma_start(out=dram_rows, in_=tile_ap)

            # Stores default to the scalar HWDGE ring; the tail stores
            # alternate onto the sync ring (empty once loads finish) so the
            # stores-only end phase runs dual-row at full DMA rate.
            HH = HWD // 2
            tail_stores = []
            for t in range(NT):
                u, b = t % 2, t // 2
                av_ap = av_sb[u][:, b : b + 1]
                rows = slice(t * 128, (t + 1) * 128)
                if t in (0, NT - 1):
                    # Quarter the first tile (small first DMAs ramp the SDMA
                    # engines faster, stores start sooner) and the last tile
                    # (short load->add->store pipeline tail after the final
                    # load, final stores split across both rings).
                    QQ = HWD // 4
                    for h in range(4):
                        quar = hpool.tile([128, QQ], F32, tag="xq", name=f"xq{t}_{h}")
                        cols = slice(h * QQ, (h + 1) * QQ)
                        nc.sync.dma_start(out=quar[:], in_=x_d[rows, cols])
                        if t == NT - 1 and h == 2:
                            nc.vector.tensor_scalar_add(
                                out=quar[:], in0=quar[:], scalar1=av_ap
                            )
                            tail_stores.append((y_d[rows, cols], quar[:]))
                        else:
                            add_store(quar[:], y_d[rows, cols], av_ap, nc.scalar)
                elif t in (12, 14):
                    # Split this store across the rings: first half to the
                    # scalar ring now, second half to the sync-ring tail.
                    tile = xpool.tile([128, HWD], F32, tag="xt")
                    nc.sync.dma_start(out=tile[:], in_=x_d[rows, :])
                    nc.vector.tensor_scalar_add(out=tile[:], in0=tile[:], scalar1=av_ap)
                    nc.scalar.dma_start(out=y_d[rows, 0:HH], in_=tile[:, 0:HH])
                    tail_stores.append((y_d[rows, HH:], tile[:, HH:]))
                else:
                    tile = xpool.tile([128, HWD], F32, tag="xt")
                    nc.sync.dma_start(out=tile[:], in_=x_d[rows, :])
                    add_store(tile[:], y_d[rows, :], av_ap, nc.scalar)
            # Issued after every load in program order -> they sit at the end
            # of the sync ring FIFO and never block a load.
            for dst, src in tail_stores:
                nc.sync.dma_start(out=dst, in_=src)

    nc.compile()
    return nc


def _prep_consts(in_proj_w, in_proj_b, out_w, out_b, kv_w, kv_b):
    import ml_dtypes

    c = C
    kvw2 = np.asarray(kv_w, np.float64)[c : 2 * c, :]        # (c, emb)
    wv = np.asarray(in_proj_w, np.float64)[2 * c :, :]       # (c, c)
    bv = np.asarray(in_proj_b, np.float64)[2 * c :]
    kvb2 = np.asarray(kv_b, np.float64)[c : 2 * c]
    ow = np.asarray(out_w, np.float64)
    M = (ow @ wv @ kvw2).astype(np.float32)                  # (c, emb)
    bconst = (ow @ (wv @ kvb2 + bv) + np.asarray(out_b, np.float64)).astype(np.float32)

    base16 = np.empty((128, C16_COLS), ml_dtypes.bfloat16)
    base16[:, M_O : M_O + 4 * c] = (
        M.T.reshape(4, 128, c).transpose(1, 0, 2).reshape(128, 4 * c)
    ).astype(ml_dtypes.bfloat16)
    base32 = np.empty((128, 2), np.float32)
    for u in range(2):
        base32[:, u] = bconst[u * 128 : (u + 1) * 128]
    return base16, base32


def make_in_maps(x, cond_emb, in_proj_w, in_proj_b, out_w, out_b, kv_w, kv_b):
    import ml_dtypes

    base16, base32 = _prep_consts(in_proj_w, in_proj_b, out_w, out_b, kv_w, kv_b)
    in_maps = []
    for r in range(NCORES):
        xs = np.ascontiguousarray(
            x[r * BS : (r + 1) * BS].reshape(ROWS, HWD), dtype=np.float32
        )
        c16 = base16.copy()
        c16[:, COND_O : COND_O + 4 * BS] = (
            cond_emb[r * BS : (r + 1) * BS]
            .T.reshape(4, 128, BS)
            .transpose(1, 0, 2)
            .reshape(128, 4 * BS)
        ).astype(ml_dtypes.bfloat16)
        in_maps.append({"x": xs, "consts16": c16, "consts32": base32})
    return in_maps


def get_nc():
    if "nc" not in _CACHE:
        _CACHE["nc"] = _build_nc()
    return _CACHE["nc"]


def kernel(x, cond_emb, ln_gamma, ln_beta, in_proj_w, in_proj_b, out_w, out_b, kv_w, kv_b):
    x = np.asarray(x, dtype=np.float32)
    nc = get_nc()
    in_maps = make_in_maps(
        x,
        np.asarray(cond_emb, np.float32),
        np.asarray(in_proj_w, np.float32),
        np.asarray(in_proj_b, np.float32),
        np.asarray(out_w, np.float32),
        np.asarray(out_b, np.float32),
        np.asarray(kv_w, np.float32),
        np.asarray(kv_b, np.float32),
    )
    res = run_bass_kernel_spmd(nc, in_maps, core_ids=list(range(NCORES)))
    y = np.empty((B, C, H, W), np.float32)
    for r in range(NCORES):
        y[r * BS : (r + 1) * BS] = res.results[r]["y"].reshape(BS, C, H, W)
    return y

